# revision 1
# baseline (speedup 1.0000x reference)
"""Co-attention kernel for Trainium2 (8 NeuronCores, data-parallel over batch).

Reference computation (B=32, L1=L2=1024, D=512):
    u  = u_fea @ Wu.T + bu            # (B, L1, D)
    i  = i_fea @ Wi.T + bi            # (B, L2, D)
    S  = (u @ M) @ i.T                # (B, L1, L2)
    u_score = S.max(axis=2); i_score = S.max(axis=1)
    p_u = softmax(u_score, axis=1)[:, :, None]
    p_i = softmax(i_score, axis=1)[:, :, None]

Algebraic refactor (constant weight folding on host):
    W3 = Wu.T @ M @ Wi          (D, D)
    b3 = bu @ M @ Wi            (D,)
    wc = Wu.T @ M @ bi          (D,)      c0 = bu @ M @ bi
    A2 = u_fea @ W3 + b3        (L, D)
    c  = u_fea @ wc + c0        (L,)      <- O(B*L*D), folded on host
    S  = A2 @ i_fea.T + c[:, None]        <- all O(B*L^2*D) work on device

Device per batch (inputs pre-transposed to feature-major on host):
    a2T[f, l] = sum_e W3[e, f] * u_feaT[e, l] + b3[f]
    S[l, m]   = sum_f a2T[f, l] * i_feaT[f, m]   (+ c[l] fused in reduction)
    row/col max + softmax on device.

Matmuls run in float32r (fp22 multiply, fp32 accumulate).
Sharding: batch 32 -> 8 cores x 4 batches each.
"""

import sys
import numpy as np

if "/opt/trn_rl_repo" not in sys.path:
    sys.path.insert(0, "/opt/trn_rl_repo")

import concourse.bass as bass  # noqa: E402,F401
import concourse.tile as tile  # noqa: E402
import concourse.mybir as mybir  # noqa: E402
from concourse import bacc, bass_isa  # noqa: E402
from concourse.bass import ts  # noqa: E402
from concourse.bass_utils import run_bass_kernel_spmd  # noqa: E402

P = 128
D = 512
L = 1024
NB = 4          # batches per core
NCORES = 8
EC = D // P     # feature chunks (4)
LJ = L // P     # l chunks (8)
F32 = mybir.dt.float32
F32R = mybir.dt.float32r
AF = mybir.ActivationFunctionType
AX = mybir.AxisListType
ALU = mybir.AluOpType

_CACHE = {}


def _build_nc(repeat=1):
    """repeat>1 re-runs the per-batch pipeline `repeat` times (same data,
    same outputs) — only used for wall-clock slope timing."""
    nc = bacc.Bacc("TRN2", target_bir_lowering=False, debug=False,
                   num_devices=NCORES)
    u4t = nc.dram_tensor("u4t", [NB, D, L], F32R, kind="ExternalInput")
    i4t = nc.dram_tensor("i4t", [NB, D, L], F32R, kind="ExternalInput")
    w3 = nc.dram_tensor("w3", [D, D], F32R, kind="ExternalInput")
    b3 = nc.dram_tensor("b3", [D], F32, kind="ExternalInput")
    cvec = nc.dram_tensor("cvec", [NB, L], F32, kind="ExternalInput")
    identm = nc.dram_tensor("identm", [P, P], F32, kind="ExternalInput")
    pu = nc.dram_tensor("pu", [NB, L], F32, kind="ExternalOutput")
    pi = nc.dram_tensor("pi", [NB, L], F32, kind="ExternalOutput")

    with tile.TileContext(nc) as tc:
        with (
            tc.tile_pool(name="const", bufs=1) as cpool,
            tc.tile_pool(name="feat", bufs=2) as feat_pool,
            tc.tile_pool(name="stg", bufs=1) as stg_pool,
            tc.tile_pool(name="acc", bufs=2) as acc_pool,
            tc.tile_pool(name="psum", bufs=1, space="PSUM") as pspool,
        ):
            w3_sb = cpool.tile([P, EC, D], F32R)
            b3_sb = cpool.tile([P, EC], F32)
            ident = cpool.tile([P, P], F32)
            w3_src = w3.ap().rearrange("(c p) d -> p c d", p=P)

            first = True
            for bb in range(NB * repeat):
                b = bb % NB
                # ---- loads (feature-major, pre-transposed on host) ----
                u_feaT = feat_pool.tile([P, EC, L], F32R, tag="u_feaT")
                i_feaT = feat_pool.tile([P, EC, L], F32R, tag="i_feaT")
                u_src = u4t.ap()[b].rearrange("(c p) l -> p c l", p=P)
                i_src = i4t.ap()[b].rearrange("(c p) l -> p c l", p=P)
                for ec in range(EC):
                    if first:
                        nc.sync.dma_start(w3_sb[:, ec, :], w3_src[:, ec, :])
                    nc.sync.dma_start(u_feaT[:, ec, ts(0, 512)],
                                      u_src[:, ec, ts(0, 512)])
                c_tile = acc_pool.tile([P, LJ], F32, tag="c_tile")
                nc.sync.dma_start(c_tile[:],
                                  cvec.ap()[b].rearrange("(j p) -> p j", p=P))
                if first:
                    first = False
                    nc.sync.dma_start(b3_sb[:],
                                      b3.ap().rearrange("(c p) -> p c", p=P))
                    nc.sync.dma_start(ident[:], identm.ap())
                for mh in range(2):
                    for ec in range(EC):
                        nc.sync.dma_start(i_feaT[:, ec, ts(mh, 512)],
                                          i_src[:, ec, ts(mh, 512)])
                for ec in range(EC):
                    nc.sync.dma_start(u_feaT[:, ec, ts(1, 512)],
                                      u_src[:, ec, ts(1, 512)])

                # ---- stage 1 (lh half) then stage 2 (j half), interleaved
                # so stage-2 j<4 runs before u's lh1 half even arrives ----
                a2T = stg_pool.tile([P, EC, L], F32R, tag="a2T", bufs=2)
                u_sc = acc_pool.tile([P, LJ], F32, tag="u_sc")
                i_acc = acc_pool.tile([P, L], F32, tag="i_acc")
                for lh in range(2):
                    for fc in range(EC):
                        ps_1 = pspool.tile([P, 512], F32, tag="ps_1", bufs=2,
                                           name=f"ps_1_{b}_{fc}_{lh}")
                        for ec in range(EC):
                            nc.tensor.matmul(
                                ps_1[:],
                                w3_sb[:, ec, ts(fc, P)],
                                u_feaT[:, ec, ts(lh, 512)],
                                start=(ec == 0), stop=(ec == EC - 1))
                        nc.scalar.activation(a2T[:, fc, ts(lh, 512)], ps_1[:],
                                             AF.Identity, bias=b3_sb[:, fc:fc + 1])
                    for j in range(lh * 4, lh * 4 + 4):
                        ps_s = pspool.tile([P, 2, 512], F32, tag="ps_s", bufs=3,
                                           name=f"ps_s_{b}_{j}")
                        for mh in range(2):
                            for fc in range(EC):
                                nc.tensor.matmul(
                                    ps_s[:, mh, :],
                                    a2T[:, fc, ts(j, P)],
                                    i_feaT[:, fc, ts(mh, 512)],
                                    start=(fc == 0), stop=(fc == EC - 1))
                        ps_flat = ps_s[:].rearrange("p a b -> p (a b)")
                        nc.vector.reduce_max(u_sc[:, j:j + 1], ps_flat,
                                             axis=AX.X)
                        if j == 0:
                            nc.vector.tensor_scalar_add(
                                i_acc[:], ps_flat, c_tile[:, j:j + 1])
                        else:
                            nc.vector.scalar_tensor_tensor(
                                i_acc[:], ps_flat, c_tile[:, j:j + 1],
                                i_acc[:], op0=ALU.add, op1=ALU.max)

                # ---- u softmax over the [P, LJ] score layout ----
                nc.vector.tensor_add(u_sc[:], u_sc[:], c_tile[:])
                u_mx = acc_pool.tile([P, 1], F32, tag="u_mx")
                nc.vector.reduce_max(u_mx[:], u_sc[:], axis=AX.X)
                u_mxg = acc_pool.tile([P, 1], F32, tag="u_mxg")
                nc.gpsimd.partition_all_reduce(u_mxg[:], u_mx[:], channels=P,
                                               reduce_op=bass_isa.ReduceOp.max)
                u_nmxg = acc_pool.tile([P, 1], F32, tag="u_nmxg")
                nc.vector.tensor_scalar_mul(u_nmxg[:], u_mxg[:], -1.0)
                u_exp = acc_pool.tile([P, LJ], F32, tag="u_exp")
                u_sum = acc_pool.tile([P, 1], F32, tag="u_sum")
                nc.scalar.activation(u_exp[:], u_sc[:], AF.Exp,
                                     bias=u_nmxg[:], accum_out=u_sum[:])
                u_zs = acc_pool.tile([P, 1], F32, tag="u_zs")
                nc.gpsimd.partition_all_reduce(u_zs[:], u_sum[:], channels=P,
                                               reduce_op=bass_isa.ReduceOp.add)
                u_rz = acc_pool.tile([P, 1], F32, tag="u_rz")
                nc.vector.reciprocal(u_rz[:], u_zs[:])
                u_p = acc_pool.tile([P, LJ], F32, tag="u_p")
                nc.vector.tensor_scalar_mul(u_p[:], u_exp[:], u_rz[:])
                nc.sync.dma_start(pu.ap()[b].rearrange("(j p) -> p j", p=P),
                                  u_p[:])

                # ---- i softmax ----
                if bb != NB * repeat - 1:
                    # mid-kernel batches: gpsimd all-reduce overlaps next batch
                    i_red = acc_pool.tile([P, L], F32, tag="i_red")
                    nc.gpsimd.partition_all_reduce(
                        i_red[:], i_acc[:], channels=P,
                        reduce_op=bass_isa.ReduceOp.max)
                    i_exp = acc_pool.tile([P, L], F32, tag="i_exp")
                    i_sum = acc_pool.tile([P, 1], F32, tag="i_sum")
                    nc.scalar.activation(i_exp[:], i_red[:], AF.Exp,
                                         bias=u_nmxg[:], accum_out=i_sum[:])
                    i_rz = acc_pool.tile([P, 1], F32, tag="i_rz")
                    nc.vector.reciprocal(i_rz[:], i_sum[:])
                    i_p = acc_pool.tile([P, L], F32, tag="i_p")
                    nc.vector.tensor_scalar_mul(i_p[:, :], i_exp[:], i_rz[:])
                    nc.sync.dma_start(pi.ap()[b].rearrange("(a m) -> a m", a=1),
                                      i_p[0:1, :])
                else:
                    # last batch: PE is idle in the tail — transpose i_acc and
                    # finish in the cheap [P, LJ] layout
                    i_sc = acc_pool.tile([P, LJ], F32, tag="i_sc")
                    for j in range(LJ):
                        ps_f = pspool.tile([P, 2, 512], F32, tag="ps_s",
                                           bufs=3,
                                           name=f"ps_f_{b}_{j}")[:, 0, :P]
                        nc.tensor.transpose(ps_f[:], i_acc[:, ts(j, P)], ident)
                        nc.vector.reduce_max(i_sc[:, j:j + 1], ps_f[:],
                                             axis=AX.X)
                    i_exp2 = acc_pool.tile([P, LJ], F32, tag="i_exp2")
                    i_sum2 = acc_pool.tile([P, 1], F32, tag="i_sum2")
                    nc.scalar.activation(i_exp2[:], i_sc[:], AF.Exp,
                                         bias=u_nmxg[:], accum_out=i_sum2[:])
                    i_zs = acc_pool.tile([P, 1], F32, tag="i_zs")
                    nc.gpsimd.partition_all_reduce(
                        i_zs[:], i_sum2[:], channels=P,
                        reduce_op=bass_isa.ReduceOp.add)
                    i_rz2 = acc_pool.tile([P, 1], F32, tag="i_rz2")
                    nc.vector.reciprocal(i_rz2[:], i_zs[:])
                    i_p2 = acc_pool.tile([P, LJ], F32, tag="i_p2")
                    nc.vector.tensor_scalar_mul(i_p2[:], i_exp2[:], i_rz2[:])
                    nc.sync.dma_start(
                        pi.ap()[b].rearrange("(j p) -> p j", p=P), i_p2[:])

    nc.compile()
    return nc


def make_in_maps(u_fea, i_fea, M, Wu, bu, Wi, bi):
    u_fea = np.asarray(u_fea, dtype=np.float32)
    i_fea = np.asarray(i_fea, dtype=np.float32)
    M64 = np.asarray(M, dtype=np.float64)
    Wu64 = np.asarray(Wu, dtype=np.float64)
    Wi64 = np.asarray(Wi, dtype=np.float64)
    bu64 = np.asarray(bu, dtype=np.float64)
    bi64 = np.asarray(bi, dtype=np.float64)

    W2 = Wu64.T @ M64                       # (D, D)
    w3 = (W2 @ Wi64).astype(np.float32)     # (D, D)  [e, f]
    b3 = ((bu64 @ M64) @ Wi64).astype(np.float32)     # (D,)
    wc = (W2 @ bi64).astype(np.float32)     # (D,)
    c0 = float((bu64 @ M64) @ bi64)
    # c = u_fea @ wc + c0 : O(B*L*D) epilogue fold, same order as input size
    cfull = (u_fea.astype(np.float64) @ wc.astype(np.float64) + c0
             ).astype(np.float32)           # (B, L)

    u_t = np.ascontiguousarray(u_fea.transpose(0, 2, 1))   # (B, D, L)
    i_t = np.ascontiguousarray(i_fea.transpose(0, 2, 1))   # (B, D, L)

    in_maps = []
    for c in range(NCORES):
        sl = slice(c * NB, (c + 1) * NB)
        in_maps.append({
            "u4t": u_t[sl], "i4t": i_t[sl],
            "w3": w3, "b3": b3, "cvec": cfull[sl],
            "identm": np.eye(P, dtype=np.float32),
        })
    return in_maps


def kernel(u_fea, i_fea, M, Wu, bu, Wi, bi):
    if "nc" not in _CACHE:
        _CACHE["nc"] = _build_nc()
    nc = _CACHE["nc"]
    in_maps = make_in_maps(u_fea, i_fea, M, Wu, bu, Wi, bi)
    res = run_bass_kernel_spmd(nc, in_maps, core_ids=list(range(NCORES)))
    _CACHE["last_results"] = res

    p_u = np.concatenate([res.results[c]["pu"] for c in range(NCORES)], axis=0)
    p_i = np.concatenate([res.results[c]["pi"] for c in range(NCORES)], axis=0)
    return p_u[:, :, None].astype(np.float32), p_i[:, :, None].astype(np.float32)



# revision 14
# speedup vs baseline: 5.5683x; 5.5683x over previous
"""Co-attention kernel for Trainium2 (8 NeuronCores, data-parallel over batch).

Reference computation (B=32, L1=L2=1024, D=512):
    u  = u_fea @ Wu.T + bu            # (B, L1, D)
    i  = i_fea @ Wi.T + bi            # (B, L2, D)
    S  = (u @ M) @ i.T                # (B, L1, L2)
    u_score = S.max(axis=2); i_score = S.max(axis=1)
    p_u = softmax(u_score, axis=1)[:, :, None]
    p_i = softmax(i_score, axis=1)[:, :, None]

Algebraic refactor (constant weight folding on host):
    W3 = Wu.T @ M @ Wi          (D, D)
    b3 = bu @ M @ Wi            (D,)
    wc = Wu.T @ M @ bi          (D,)      c0 = bu @ M @ bi
    A2 = u_fea @ W3 + b3        (L, D)
    c  = u_fea @ wc + c0        (L,)      <- O(B*L*D), folded on host
    S  = A2 @ i_fea.T + c[:, None]        <- all O(B*L^2*D) work on device

Device per batch (inputs pre-transposed to feature-major on host):
    a2T[f, l] = sum_e W3[e, f] * u_feaT[e, l] + b3[f]
    S[l, m]   = sum_f a2T[f, l] * i_feaT[f, m]   (+ c[l] fused in reduction)
    row/col max + softmax on device.

Matmuls run in float32r (fp22 multiply, fp32 accumulate).
Sharding: batch 32 -> 8 cores x 4 batches each.
"""

import sys
import numpy as np

if "/opt/trn_rl_repo" not in sys.path:
    sys.path.insert(0, "/opt/trn_rl_repo")

import concourse.bass as bass  # noqa: E402,F401
import concourse.tile as tile  # noqa: E402
import concourse.mybir as mybir  # noqa: E402
from concourse import bacc, bass_isa  # noqa: E402
from concourse.bass import ts  # noqa: E402
from concourse.bass_utils import run_bass_kernel_spmd  # noqa: E402

P = 128
D = 512
L = 1024
NB = 4          # batches per core
NCORES = 8
EC = D // P     # feature chunks (4)
LJ = L // P     # l chunks (8)
F32 = mybir.dt.float32
F16 = mybir.dt.float16
F32R = mybir.dt.float32r
SHIFT = 140.0   # softmax-stability shift folded into cvec (softmax is
                # shift-invariant; keeps fp16 score quantization fine-grained
                # near the top of each softmax group)
AF = mybir.ActivationFunctionType
AX = mybir.AxisListType
ALU = mybir.AluOpType

_CACHE = {}


def _build_nc(repeat=1):
    """repeat>1 re-runs the per-batch pipeline `repeat` times (same data,
    same outputs) — only used for wall-clock slope timing."""
    nc = bacc.Bacc("TRN2", target_bir_lowering=False, debug=False,
                   num_devices=NCORES)
    u4t = nc.dram_tensor("u4t", [NB, D, L], F32R, kind="ExternalInput")
    i4t = nc.dram_tensor("i4t", [NB, D, L], F32R, kind="ExternalInput")
    w3 = nc.dram_tensor("w3", [D, D], F32R, kind="ExternalInput")
    b3 = nc.dram_tensor("b3", [D], F32, kind="ExternalInput")
    cvec = nc.dram_tensor("cvec", [NB, L], F32, kind="ExternalInput")
    pu = nc.dram_tensor("pu", [NB, L], F32, kind="ExternalOutput")
    pi = nc.dram_tensor("pi", [NB, L], F32, kind="ExternalOutput")

    with tile.TileContext(nc) as tc:
        with (
            tc.tile_pool(name="const", bufs=1) as cpool,
            tc.tile_pool(name="feat", bufs=2) as feat_pool,
            tc.tile_pool(name="stg", bufs=1) as stg_pool,
            tc.tile_pool(name="acc", bufs=2) as acc_pool,
            tc.tile_pool(name="sc", bufs=3) as sc_pool,
            tc.tile_pool(name="psum", bufs=1, space="PSUM") as pspool,
        ):
            w3_sb = cpool.tile([P, EC, D], F32R)
            b3_sb = cpool.tile([P, EC], F32)
            w3_src = w3.ap().rearrange("(c p) d -> p c d", p=P)

            first = True
            for bb in range(NB * repeat):
                b = bb % NB
                # ---- loads (feature-major, pre-transposed on host) ----
                u_feaT = feat_pool.tile([P, EC, L], F32R, tag="u_feaT")
                i_feaT = feat_pool.tile([P, EC, L], F32R, tag="i_feaT")
                u_src = u4t.ap()[b].rearrange("(c p) l -> p c l", p=P)
                i_src = i4t.ap()[b].rearrange("(c p) l -> p c l", p=P)
                for ec in range(EC):
                    if first:
                        nc.sync.dma_start(w3_sb[:, ec, :], w3_src[:, ec, :])
                    nc.sync.dma_start(u_feaT[:, ec, ts(0, 512)],
                                      u_src[:, ec, ts(0, 512)])
                c_tile = acc_pool.tile([P, LJ], F32, tag="c_tile")
                nc.sync.dma_start(c_tile[:],
                                  cvec.ap()[b].rearrange("(j p) -> p j", p=P))
                if first:
                    first = False
                    nc.sync.dma_start(b3_sb[:],
                                      b3.ap().rearrange("(c p) -> p c", p=P))
                for mh in range(2):
                    for ec in range(EC):
                        nc.sync.dma_start(i_feaT[:, ec, ts(mh, 512)],
                                          i_src[:, ec, ts(mh, 512)])
                for ec in range(EC):
                    nc.sync.dma_start(u_feaT[:, ec, ts(1, 512)],
                                      u_src[:, ec, ts(1, 512)])

                # ---- stage 1 (lh half) then stage 2 (j half), interleaved
                # so stage-2 j<4 runs before u's lh1 half even arrives ----
                a2T = stg_pool.tile([P, EC, L], F32R, tag="a2T", bufs=2)
                u_sc = acc_pool.tile([P, LJ], F32, tag="u_sc")
                i_acc = acc_pool.tile([P, L], F16, tag="i_acc")
                for lh in range(2):
                    for fc in range(EC):
                        ps_1 = pspool.tile([P, 512], F32, tag="ps_1", bufs=2,
                                           name=f"ps_1_{b}_{fc}_{lh}")
                        for ec in range(EC):
                            nc.tensor.matmul(
                                ps_1[:],
                                w3_sb[:, ec, ts(fc, P)],
                                u_feaT[:, ec, ts(lh, 512)],
                                start=(ec == 0), stop=(ec == EC - 1))
                        nc.scalar.activation(a2T[:, fc, ts(lh, 512)], ps_1[:],
                                             AF.Identity, bias=b3_sb[:, fc:fc + 1])
                    for j in range(lh * 4, lh * 4 + 4):
                        ps_s = pspool.tile([P, 2, 512], F32, tag="ps_s", bufs=3,
                                           name=f"ps_s_{b}_{j}")
                        for mh in range(2):
                            for fc in range(EC):
                                nc.tensor.matmul(
                                    ps_s[:, mh, :],
                                    a2T[:, fc, ts(j, P)],
                                    i_feaT[:, fc, ts(mh, 512)],
                                    start=(fc == 0), stop=(fc == EC - 1))
                        ps_flat = ps_s[:].rearrange("p a b -> p (a b)")
                        # Act drains PSUM once, fusing +c[l]-SHIFT and casting
                        # to fp16; both DVE max passes then run at 2x on fp16.
                        s_c = sc_pool.tile([P, L], F16, tag="s_c",
                                           name=f"s_c_{b}_{j}")
                        nc.scalar.activation(s_c[:], ps_flat, AF.Identity,
                                             bias=c_tile[:, j:j + 1])
                        nc.vector.reduce_max(u_sc[:, j:j + 1], s_c[:],
                                             axis=AX.X)
                        if j == 0:
                            nc.vector.tensor_copy(i_acc[:], s_c[:])
                        else:
                            nc.vector.tensor_max(i_acc[:], s_c[:], i_acc[:])

                # ---- u softmax over the [P, LJ] score layout ----
                u_mx = acc_pool.tile([P, 1], F32, tag="u_mx")
                nc.vector.reduce_max(u_mx[:], u_sc[:], axis=AX.X)
                u_mxg = acc_pool.tile([P, 1], F32, tag="u_mxg")
                nc.gpsimd.partition_all_reduce(u_mxg[:], u_mx[:], channels=P,
                                               reduce_op=bass_isa.ReduceOp.max)
                u_nmxg = acc_pool.tile([P, 1], F32, tag="u_nmxg")
                nc.vector.tensor_scalar_mul(u_nmxg[:], u_mxg[:], -1.0)
                u_exp = acc_pool.tile([P, LJ], F32, tag="u_exp")
                u_sum = acc_pool.tile([P, 1], F32, tag="u_sum")
                nc.scalar.activation(u_exp[:], u_sc[:], AF.Exp,
                                     bias=u_nmxg[:], accum_out=u_sum[:])
                u_zs = acc_pool.tile([P, 1], F32, tag="u_zs")
                nc.gpsimd.partition_all_reduce(u_zs[:], u_sum[:], channels=P,
                                               reduce_op=bass_isa.ReduceOp.add)
                u_rz = acc_pool.tile([P, 1], F32, tag="u_rz")
                nc.vector.reciprocal(u_rz[:], u_zs[:])
                u_p = acc_pool.tile([P, LJ], F32, tag="u_p")
                nc.vector.tensor_scalar_mul(u_p[:], u_exp[:], u_rz[:])
                nc.sync.dma_start(pu.ap()[b].rearrange("(j p) -> p j", p=P),
                                  u_p[:])

                # ---- i softmax (gpsimd all-reduce overlaps next batch) ----
                i_red = acc_pool.tile([P, L], F32, tag="i_red")
                nc.gpsimd.partition_all_reduce(
                    i_red[:], i_acc[:], channels=P,
                    reduce_op=bass_isa.ReduceOp.max)
                i_exp = acc_pool.tile([P, L], F32, tag="i_exp")
                i_sum = acc_pool.tile([P, 1], F32, tag="i_sum")
                nc.scalar.activation(i_exp[:], i_red[:], AF.Exp,
                                     bias=u_nmxg[:], accum_out=i_sum[:])
                i_rz = acc_pool.tile([P, 1], F32, tag="i_rz")
                nc.vector.reciprocal(i_rz[:], i_sum[:])
                i_p = acc_pool.tile([P, L], F32, tag="i_p")
                nc.vector.tensor_scalar_mul(i_p[:, :], i_exp[:], i_rz[:])
                nc.sync.dma_start(pi.ap()[b].rearrange("(a m) -> a m", a=1),
                                  i_p[0:1, :])

    nc.compile()
    return nc


def make_in_maps(u_fea, i_fea, M, Wu, bu, Wi, bi):
    u_fea = np.asarray(u_fea, dtype=np.float32)
    i_fea = np.asarray(i_fea, dtype=np.float32)
    M64 = np.asarray(M, dtype=np.float64)
    Wu64 = np.asarray(Wu, dtype=np.float64)
    Wi64 = np.asarray(Wi, dtype=np.float64)
    bu64 = np.asarray(bu, dtype=np.float64)
    bi64 = np.asarray(bi, dtype=np.float64)

    W2 = Wu64.T @ M64                       # (D, D)
    w3 = (W2 @ Wi64).astype(np.float32)     # (D, D)  [e, f]
    b3 = ((bu64 @ M64) @ Wi64).astype(np.float32)     # (D,)
    wc = (W2 @ bi64).astype(np.float32)     # (D,)
    c0 = float((bu64 @ M64) @ bi64)
    # c = u_fea @ wc + c0 : O(B*L*D) epilogue fold, same order as input size.
    # SHIFT keeps the fp16 on-device scores small near the softmax top;
    # softmax output is invariant to it.
    cfull = (u_fea.astype(np.float64) @ wc.astype(np.float64) + c0 - SHIFT
             ).astype(np.float32)           # (B, L)

    u_t = np.ascontiguousarray(u_fea.transpose(0, 2, 1))   # (B, D, L)
    i_t = np.ascontiguousarray(i_fea.transpose(0, 2, 1))   # (B, D, L)

    in_maps = []
    for c in range(NCORES):
        sl = slice(c * NB, (c + 1) * NB)
        in_maps.append({
            "u4t": u_t[sl], "i4t": i_t[sl],
            "w3": w3, "b3": b3, "cvec": cfull[sl],
        })
    return in_maps


def kernel(u_fea, i_fea, M, Wu, bu, Wi, bi):
    if "nc" not in _CACHE:
        _CACHE["nc"] = _build_nc()
    nc = _CACHE["nc"]
    in_maps = make_in_maps(u_fea, i_fea, M, Wu, bu, Wi, bi)
    res = run_bass_kernel_spmd(nc, in_maps, core_ids=list(range(NCORES)))
    _CACHE["last_results"] = res

    p_u = np.concatenate([res.results[c]["pu"] for c in range(NCORES)], axis=0)
    p_i = np.concatenate([res.results[c]["pi"] for c in range(NCORES)], axis=0)
    return p_u[:, :, None].astype(np.float32), p_i[:, :, None].astype(np.float32)

